# revision 36
# baseline (speedup 1.0000x reference)
"""RGCN (2-layer, per-(dst,rel) mean aggregation) + triplet projection on 8
Trainium2 NeuronCores — v3: host-prepared layer-1 stream, sender-side
compaction + AllToAll for layer 2, per-node triplet outputs.

Data flow per core:
- Layer 1: message stream fully host-prepared (t1msg[slot] = norm_e *
  x[src_e], partition-major) -> contiguous static reads, slab one-hot
  aggregation matmuls, W-stationary apply -> h1T in SBUF + node-major h1
  rows written per half (h1own_h0 / h1own_h1).
- Sender-side compaction: each core gathers from its LOCAL h1own the rows
  each peer needs (int16-safe, 3328-row half blocks), packed into
  per-(dest, half) 2048-row cells.  Half-0 cells gather DURING layer 1
  (windows 0-25 done), half-1 right after.  Two AllToAlls deliver each
  core's compacted gather table t2 [2, 8, 2048, F] (= 32768 rows, int16).
- Layer 2: 52 x 1024-row dma_gather stream pieces from t2, same agg/apply,
  then per-node u = h2 @ Wpu and v = h2 @ Wpv written as outputs.
- Host: out[e] = u[src_e] + v[dst_e] + bp.
"""

import numpy as np
import ml_dtypes

BF16 = ml_dtypes.bfloat16

N, R, F, E, NCORES = 50000, 8, 256, 400000, 8
W = 52                   # windows per core
HW_ = 26                 # windows per half
NCAP = W * 128           # 6656 node slots per core
CHUNKS = W * R           # 416 msg chunks per core per layer
SLOTS = CHUNKS * 128     # 53248 msg stream slots
PIECE = 1024             # rows per msg dma_gather piece
GRP = 4                  # windows per apply group (512 dst cols)
NGRP = W // GRP          # 13
SCELL = 2048             # rows per (dest, half) send cell
TBL = 2 * NCORES * SCELL  # 32768 compacted table rows (int16 ceiling)
LAST_EXEC_NS = None
LAST_RES = None
LAST_PLAN = None


# ---------------------------------------------------------------- planning

def _pack_nodes(src, dst, et, rng):
    """Assign nodes to (core, window, slot).

    Constraints: per-(window, rel) in-degree <= 128, <= 128 nodes/window.
    Balance objective: per owner d, split each dest-core c's unique-src set
    evenly across window halves (cells must fit SCELL rows).
    """
    deg = np.zeros((N, R), dtype=np.int64)
    np.add.at(deg, (dst, et), 1)
    perm = rng.permutation(N)
    core_of = np.zeros(N, dtype=np.int64)
    base = N // NCORES
    for c in range(NCORES):
        core_of[perm[c * base:(c + 1) * base]] = c

    # vec(n): which dest cores n has out-edges to (unique-src membership)
    ecore = core_of[dst]
    vec = np.zeros((N, NCORES), dtype=bool)
    vec[src, ecore] = True

    win_of = np.zeros(N, dtype=np.int64)
    slot_of = np.zeros(N, dtype=np.int64)
    for c in range(NCORES):
        nodes = np.where(core_of == c)[0]
        dv = deg[nodes]
        order = np.argsort(-dv.max(axis=1), kind="stable")
        nodes, dv = nodes[order], dv[order]
        load = np.zeros((W, R), dtype=np.int64)
        cnt = np.zeros(W, dtype=np.int64)
        hcnt = np.zeros((NCORES, 2), dtype=np.int64)
        wsel = np.zeros(len(nodes), dtype=np.int64)
        half_of_w = (np.arange(W) >= HW_).astype(np.int64)
        for i in range(len(nodes)):
            n = nodes[i]
            v = dv[i]
            after = load + v
            feas = (after <= 128).all(axis=1) & (cnt < 128)
            if not feas.any():
                feas = cnt < 128
            # balance penalty: prefer the half where n's dest cores are behind
            vs = vec[n]
            diff = hcnt[vs, 0] - hcnt[vs, 1]  # >0: half0 ahead
            pen0 = float((diff > 0).sum())
            pen1 = float((diff < 0).sum())
            pen = np.where(half_of_w == 0, pen0, pen1)
            score = np.where(feas,
                             after.max(axis=1) + cnt * 0.001 + pen * 0.45,
                             np.inf)
            w = int(np.argmin(score))
            wsel[i] = w
            load[w] += v
            cnt[w] += 1
            hcnt[vs, half_of_w[w]] += 1
        win_of[nodes] = wsel
        for w in range(W):
            sel = nodes[wsel == w]
            slot_of[sel] = np.arange(len(sel))
    return core_of, win_of, slot_of, core_of * NCAP + win_of * 128 + slot_of


def _wrap16(a, total):
    w = np.asarray(a, dtype=np.int16).reshape(total // 16, 16).T
    return np.tile(w, (8, 1))


def _plan(src, dst, et, norm):
    for seed in (1234, 77, 2025, 9001, 31337):
        rng = np.random.default_rng(seed)
        p = _plan_try(src, dst, et, norm, rng)
        if p is not None:
            return p
    raise RuntimeError("could not pack cells within SCELL for any seed")


def _plan_try(src, dst, et, norm, rng):
    core_of, win_of, slot_of, gpos = _pack_nodes(src, dst, et, rng)
    ecore = core_of[dst]
    run_of_edge = win_of[dst] * R + et

    stream_eid, stream_srcn, stream_dstl, stream_norm = [], [], [], []
    for c in range(NCORES):
        eid = np.where(ecore == c)[0]
        runs = run_of_edge[eid]
        cnts = np.bincount(runs, minlength=CHUNKS)
        if cnts.max() > 128:
            return None
        order = np.argsort(runs, kind="stable")
        eid, runs = eid[order], runs[order]
        offs = np.zeros(len(eid), dtype=np.int64)
        b = np.flatnonzero(np.diff(runs)) + 1
        offs[b] = np.arange(len(eid))[b]
        offs = np.maximum.accumulate(offs)
        pos = runs * 128 + (np.arange(len(eid)) - offs)
        sn = np.full(SLOTS, -1, dtype=np.int64)   # src NODE id per slot
        ei = np.full(SLOTS, -1, dtype=np.int64)
        dl = np.zeros(SLOTS, dtype=np.int64)
        nm = np.zeros(SLOTS, dtype=np.float32)
        sn[pos] = src[eid]
        ei[pos] = eid
        dl[pos] = slot_of[dst[eid]]
        nm[pos] = norm[eid]
        stream_eid.append(ei)
        stream_srcn.append(sn)
        stream_dstl.append(dl)
        stream_norm.append(nm)

    # sender cells: for (dest c, owner d, half h): unique src nodes
    half_of = (win_of >= HW_).astype(np.int64)
    cell_nodes = [[[None] * 2 for _ in range(NCORES)] for _ in range(NCORES)]
    maxcnt = 0
    for c in range(NCORES):
        sn = stream_srcn[c]
        u = np.unique(sn[sn >= 0])
        od, oh = core_of[u], half_of[u]
        for d in range(NCORES):
            for h in range(2):
                sel = u[(od == d) & (oh == h)]
                # sender gather order: by (window, slot)
                keys = win_of[sel] * 128 + slot_of[sel]
                sel = sel[np.argsort(keys)]
                cell_nodes[c][d][h] = sel
                maxcnt = max(maxcnt, len(sel))
    if maxcnt > SCELL:
        return None

    # t2 position per (consumer c, src node), quarter-split A2A layout:
    # t2[h, j, d, i] -> flat h*16384 + j*8192 + d*1024 + i, where the
    # sender cell rank r maps to (j, i) = divmod(r, 1024)
    msg_idx, dstl_arr, nrm_arr = [], [], []
    tpos_all = []
    for c in range(NCORES):
        tpos = np.zeros(N, dtype=np.int64)
        for d in range(NCORES):
            for h in range(2):
                sel = cell_nodes[c][d][h]
                r = np.arange(len(sel))
                tpos[sel] = (h * (NCORES * SCELL) + (r // PIECE) * 8192 +
                             d * PIECE + r % PIECE)
        tpos_all.append(tpos)
        sn = stream_srcn[c]
        mi = np.zeros(SLOTS, dtype=np.int64)
        valid = sn >= 0
        mi[valid] = tpos[sn[valid]]
        msg_idx.append(_wrap16(mi, SLOTS))
        dstl_arr.append(np.ascontiguousarray(
            stream_dstl[c].reshape(CHUNKS, 128).T.astype(np.int32)))
        nrm_arr.append(np.ascontiguousarray(
            stream_norm[c].reshape(CHUNKS, 128).T))

    # sender gather idx per core d: [2 halves][8 dests][SCELL] into
    # h1own_h{h} pm flat rows (s*26 + (w - 26h))
    cell_idx = []
    for d in range(NCORES):
        ci = np.zeros((2, NCORES, SCELL), dtype=np.int64)
        for h in range(2):
            for c in range(NCORES):
                sel = cell_nodes[c][d][h]
                ci[h, c, :len(sel)] = slot_of[sel] * HW_ + (win_of[sel] - HW_ * h)
        cell_idx.append(_wrap16(ci.ravel(), 2 * NCORES * SCELL))

    inv = np.zeros(NCORES * NCAP, dtype=np.int64)
    inv[gpos] = np.arange(N)
    filled = np.zeros(NCORES * NCAP, dtype=bool)
    filled[gpos] = True
    return dict(
        gpos=gpos, inv=inv, filled=filled, core_of=core_of, win_of=win_of,
        slot_of=slot_of, msg_idx=msg_idx, dstl=dstl_arr, nrm=nrm_arr,
        cell_idx=cell_idx, stream_eid=stream_eid, stream_srcn=stream_srcn,
        maxcnt=maxcnt,
    )


# ------------------------------------------------------------------ device

def _build(dbg=False):
    import concourse.bass as bass
    import concourse.bacc as bacc
    import concourse.mybir as mybir
    import concourse.tile as tile
    from concourse.masks import make_identity

    dt = mybir.dt
    AF = mybir.ActivationFunctionType
    nc = bacc.Bacc("TRN2", target_bir_lowering=False, debug=False,
                   num_devices=NCORES, num_swdge_queues=4,
                   dynamic_dma_scratch_size=32768)

    t1msgd = nc.dram_tensor("t1msg", [128, SLOTS // 128, F], dt.bfloat16,
                            kind="ExternalInput")
    xtd = nc.dram_tensor("xt", [128, 2, NCAP], dt.bfloat16, kind="ExternalInput")
    w1d = nc.dram_tensor("w1s", [128, R, 2, 2, 128], dt.bfloat16, kind="ExternalInput")
    w2d = nc.dram_tensor("w2s", [128, R, 2, 2, 128], dt.bfloat16, kind="ExternalInput")
    r1d = nc.dram_tensor("r1s", [128, 2, 2, 128], dt.bfloat16, kind="ExternalInput")
    r2d = nc.dram_tensor("r2s", [128, 2, 2, 128], dt.bfloat16, kind="ExternalInput")
    wpud = nc.dram_tensor("wpu", [128, 2, F], dt.bfloat16, kind="ExternalInput")
    wpvd = nc.dram_tensor("wpv", [128, 2, F], dt.bfloat16, kind="ExternalInput")
    b1d = nc.dram_tensor("b1c", [128, 2], dt.float32, kind="ExternalInput")
    b2d = nc.dram_tensor("b2c", [128, 2], dt.float32, kind="ExternalInput")
    mid = nc.dram_tensor("mi", [128, SLOTS // 16], dt.int16, kind="ExternalInput")
    dstld = nc.dram_tensor("dstl", [128, CHUNKS], dt.int32, kind="ExternalInput")
    nrmd = nc.dram_tensor("nrm", [128, CHUNKS], dt.float32, kind="ExternalInput")
    cid = nc.dram_tensor("ci", [128, 2 * NCORES * SCELL // 16], dt.int16,
                         kind="ExternalInput")
    iotad = nc.dram_tensor("iota", [128, 128], dt.int32, kind="ExternalInput")
    # transposed: [fout-half-part, fout-half, node-col]
    uod = nc.dram_tensor("uo", [128, 2, NCAP], dt.bfloat16, kind="ExternalOutput")
    vod = nc.dram_tensor("vo", [128, 2, NCAP], dt.bfloat16, kind="ExternalOutput")
    if dbg:
        h1dbg = nc.dram_tensor("h1dbg", [2, 128, HW_, F], dt.bfloat16,
                               kind="ExternalOutput")
        t2dbg = nc.dram_tensor("t2dbg", [2, 2, NCORES, PIECE, F], dt.bfloat16,
                               kind="ExternalOutput")

    rg = [list(range(NCORES))]

    with tile.TileContext(nc) as tc:
        with (
            tc.tile_pool(name="const", bufs=1) as cp,
            tc.tile_pool(name="big", bufs=1) as bigp,
            tc.tile_pool(name="msg", bufs=6) as msgp,
            tc.tile_pool(name="slab", bufs=4) as slabp,
            tc.tile_pool(name="sl2", bufs=11) as slp2,
            tc.tile_pool(name="ybuf", bufs=2) as ybp,
            tc.tile_pool(name="small", bufs=3) as sp,
            tc.tile_pool(name="h2t", bufs=2) as h2p,
            tc.tile_pool(name="cell", bufs=4) as cellp,
            tc.tile_pool(name="psy", bufs=2, space="PSUM") as psyp,
            tc.tile_pool(name="work", bufs=3, space="PSUM") as workp,
            tc.tile_pool(name="pstp", bufs=1, space="PSUM") as pstp,
            tc.tile_pool(name="dram", bufs=1, space="DRAM") as dram,
        ):
            # ---- constants
            w_sb = [cp.tile([128, R, 2, 2, 128], dt.bfloat16, tag=f"w{i}", name=f"w{i}")
                    for i in range(2)]
            nc.sync.dma_start(w_sb[0][:], w1d[:])
            nc.scalar.dma_start(w_sb[1][:], w2d[:])
            r_sb = [cp.tile([128, 2, 2, 128], dt.bfloat16, tag=f"r{i}", name=f"r{i}")
                    for i in range(2)]
            nc.sync.dma_start(r_sb[0][:], r1d[:])
            nc.scalar.dma_start(r_sb[1][:], r2d[:])
            wpu_sb = cp.tile([128, 2, F], dt.bfloat16, tag="wpu", name="wpu")
            wpv_sb = cp.tile([128, 2, F], dt.bfloat16, tag="wpv", name="wpv")
            nc.scalar.dma_start(wpu_sb[:], wpud[:])
            nc.scalar.dma_start(wpv_sb[:], wpvd[:])
            b_sb = [cp.tile([128, 2], dt.float32, tag=f"b{i}", name=f"b{i}")
                    for i in range(2)]
            nc.sync.dma_start(b_sb[0][:], b1d[:])
            nc.scalar.dma_start(b_sb[1][:], b2d[:])
            mi_sb = cp.tile([128, SLOTS // 16], dt.int16, tag="mi", name="mi")
            nc.scalar.dma_start(mi_sb[:], mid[:])
            dstl_sb = cp.tile([128, CHUNKS], dt.int32, tag="dstl", name="dstl")
            nc.sync.dma_start(dstl_sb[:], dstld[:])
            nrm_sb = cp.tile([128, CHUNKS], dt.float32, tag="nrm", name="nrm")
            nc.sync.dma_start(nrm_sb[:], nrmd[:])
            ci_sb = cp.tile([128, 2 * NCORES * SCELL // 16], dt.int16,
                            tag="ci", name="ci")
            nc.scalar.dma_start(ci_sb[:], cid[:])
            iota_sb = cp.tile([128, 128], dt.int32, tag="iota", name="iota")
            nc.sync.dma_start(iota_sb[:], iotad[:])
            ident = cp.tile([128, 128], dt.bfloat16, tag="id", name="id")
            make_identity(nc, ident)

            h1T_sb = bigp.tile([128, 2, NCAP], dt.bfloat16, tag="h1T", name="h1T")

            # ---- DRAM scratch
            h1own = [dram.tile([128, HW_, F], dt.bfloat16, tag=f"h1own{h}",
                               name=f"h1own{h}") for h in range(2)]
            snd = [[dram.tile([NCORES, PIECE, F], dt.bfloat16,
                              tag=f"snd{h}{j}", name=f"snd{h}{j}")
                    for j in range(SCELL // PIECE)] for h in range(2)]
            t2 = dram.tile([2, SCELL // PIECE, NCORES, PIECE, F], dt.bfloat16,
                           tag="t2", name="t2")

            def send_piece(h, j, c, qoff, weng):
                """Gather piece (dest c, quarter j) of half h from h1own[h]."""
                src_ap = h1own[h][:].rearrange("p w f -> (p w) f")
                gt = cellp.tile([128, PIECE // 128, F], dt.bfloat16,
                                tag="ct", name=f"ct{h}_{c}{j}")
                base = (h * NCORES * SCELL + c * SCELL + j * PIECE) // 16
                nc.gpsimd.dma_gather(
                    out_ap=gt[:], in_ap=src_ap,
                    idxs_ap=ci_sb[:, base:base + PIECE // 16],
                    num_idxs=PIECE, num_idxs_reg=PIECE, elem_size=F,
                    queue_num=(j * NCORES + c + qoff) % 4)
                # snd[h][j] pm view: [128, NCORES*(PIECE//128), F]
                weng.dma_start(
                    snd[h][j][:].rearrange("c (b p) f -> p (c b) f", p=128)
                    [:, c * (PIECE // 128):(c + 1) * (PIECE // 128), :],
                    gt[:])

            def a2a(h, j):
                nc.gpsimd.collective_compute(
                    "AllToAll", mybir.AluOpType.bypass, replica_groups=rg,
                    ins=[snd[h][j][:].opt()], outs=[t2[h, j].opt()])

            sl2_tiles = {}

            def build_sl2(w):
                nb = PIECE // 128
                eq = slabp.tile([128, nb, 128], dt.bfloat16, tag="eq2",
                                name=f"eq2_{w}")
                nc.vector.tensor_tensor(
                    eq[:],
                    dstl_sb[:, w * nb:(w + 1) * nb].to_broadcast((128, nb, 128)),
                    iota_sb[:].rearrange("q (o d) -> q o d", o=1)
                    .to_broadcast((128, nb, 128)),
                    op=mybir.AluOpType.is_equal)
                sl = slp2.tile([128, nb, 128], dt.bfloat16, tag="sl2",
                               name=f"sl2_{w}")
                nc.vector.tensor_tensor(
                    sl[:], eq[:],
                    nrm_sb[:, w * nb:(w + 1) * nb].to_broadcast((128, nb, 128)),
                    op=mybir.AluOpType.mult)
                sl2_tiles[w] = sl

            def msg_piece(li, w, qoff):
                nb = PIECE // 128
                mt = msgp.tile([128, nb, F], dt.bfloat16, tag="mt",
                               name=f"mt{li}_{w}")
                if li == 0:
                    eng = nc.sync if w % 2 == 0 else nc.scalar
                    eng.dma_start(
                        mt[:], t1msgd[:, w * nb:(w + 1) * nb, :])
                else:
                    nc.gpsimd.dma_gather(
                        out_ap=mt[:],
                        in_ap=t2[:].rearrange("h j c i f -> (h j c i) f"),
                        idxs_ap=mi_sb[:, w * (PIECE // 16):(w + 1) * (PIECE // 16)],
                        num_idxs=PIECE, num_idxs_reg=PIECE, elem_size=F,
                        queue_num=(w + qoff) % 4)
                if li == 0:
                    eq = slabp.tile([128, nb, 128], dt.bfloat16, tag="eq",
                                    name=f"eq{li}_{w}")
                    nc.vector.tensor_tensor(
                        eq[:],
                        dstl_sb[:, w * nb:(w + 1) * nb]
                        .to_broadcast((128, nb, 128)),
                        iota_sb[:].rearrange("q (o d) -> q o d", o=1)
                        .to_broadcast((128, nb, 128)),
                        op=mybir.AluOpType.is_equal)
                    return mt, eq
                sl = sl2_tiles.pop(w)
                if w + 10 < W:
                    build_sl2(w + 10)
                return mt, sl

            def layer(li, rootT, qoff, mid_hook=None):
                for g in range(NGRP):
                    if mid_hook is not None:
                        mid_hook(g)
                    yb = ybp.tile([128, 2, R, 512], dt.bfloat16, tag="yb",
                                  name=f"yb{li}{g}")
                    if li == 0:
                        xg = sp.tile([128, 2, 512], dt.bfloat16, tag="xg",
                                     name=f"xg{g}")
                        nc.scalar.dma_start(xg[:], xtd[:, :, g * 512:(g + 1) * 512])
                    for wl in range(GRP):
                        w = g * GRP + wl
                        mt, sl = msg_piece(li, w, qoff)
                        for fh in range(2):
                            psY = psyp.tile([128, 1024], dt.float32, tag="psY",
                                            name=f"psY{li}{w}{fh}")
                            for r in range(R):
                                nc.tensor.matmul(
                                    psY[:, r * 128:(r + 1) * 128],
                                    lhsT=mt[:, r, fh * 128:(fh + 1) * 128],
                                    rhs=sl[:, r, :],
                                    start=True, stop=True)
                            psYr = psY[:].rearrange("q (r d) -> q r d", r=R)
                            if fh == 0:
                                nc.vector.tensor_copy(
                                    yb[:, fh, :, wl * 128:(wl + 1) * 128], psYr)
                            else:
                                nc.scalar.copy(
                                    yb[:, fh, :, wl * 128:(wl + 1) * 128], psYr)
                    for oh in range(2):
                        psA = workp.tile([128, 512], dt.float32, tag="pa",
                                         name=f"psA{li}{g}{oh}")
                        for r in range(R):
                            for fh in range(2):
                                nc.tensor.matmul(
                                    psA[:], lhsT=w_sb[li][:, r, fh, oh, :],
                                    rhs=yb[:, fh, r, :],
                                    start=(r == 0 and fh == 0), stop=False)
                        for fh in range(2):
                            rt_ap = (xg[:, fh, :] if li == 0 else
                                     rootT[:, fh, g * 512:(g + 1) * 512])
                            nc.tensor.matmul(
                                psA[:], lhsT=r_sb[li][:, fh, oh, :],
                                rhs=rt_ap,
                                start=False, stop=(fh == 1))
                        if li == 0:
                            nc.scalar.activation(
                                h1T_sb[:, oh, g * 512:(g + 1) * 512], psA[:],
                                AF.Relu, bias=b_sb[0][:, oh:oh + 1])
                        else:
                            h2t = (h2p.tile([128, 2, 512], dt.bfloat16, tag="h2t",
                                            name=f"h2t{g}")
                                   if oh == 0 else h2t)
                            nc.scalar.activation(
                                h2t[:, oh, :], psA[:], AF.Relu,
                                bias=b_sb[1][:, oh:oh + 1])
                    if li == 0:
                        for dsub in range(GRP):
                            w = g * GRP + dsub
                            h = int(w >= HW_)
                            hr = sp.tile([128, F], dt.bfloat16, tag="hr",
                                         name=f"hr{g}{dsub}")
                            for oh in range(2):
                                pst = pstp.tile([128, 128], dt.bfloat16,
                                                tag="pst", name=f"pst{g}{dsub}{oh}")
                                nc.tensor.transpose(
                                    pst[:],
                                    h1T_sb[:, oh, w * 128:(w + 1) * 128],
                                    ident[:])
                                nc.vector.tensor_copy(
                                    hr[:, oh * 128:(oh + 1) * 128], pst[:])
                            nc.sync.dma_start(
                                h1own[h][:, w - HW_ * h, :], hr[:])
                    else:
                        # transposed u/v: uT[fo-half, node] = Wpu^T @ h2^T
                        # with 512-wide rhs (whole group of nodes at once)
                        for li_uv, (dst, wmat) in enumerate(
                                ((uod, wpu_sb), (vod, wpv_sb))):
                            for fo in range(2):
                                psU = workp.tile([128, 512], dt.float32,
                                                 tag="pa",
                                                 name=f"psU{g}_{li_uv}{fo}")
                                for oh in range(2):
                                    nc.tensor.matmul(
                                        psU[:],
                                        lhsT=wmat[:, oh, fo * 128:(fo + 1) * 128],
                                        rhs=h2t[:, oh, :],
                                        start=(oh == 0), stop=(oh == 1))
                                uo = sp.tile([128, 512], dt.bfloat16, tag="uo",
                                             name=f"uo{g}_{li_uv}{fo}")
                                if fo == 0:
                                    nc.scalar.copy(uo[:], psU[:])
                                else:
                                    nc.vector.tensor_copy(uo[:], psU[:])
                                eng = nc.sync if dst is uod else nc.scalar
                                eng.dma_start(
                                    dst[:, fo, g * 512:(g + 1) * 512], uo[:])

            # h0 cell pieces interleaved into groups 8-12, j-major so the
            # first sub-A2A can fire while quarter-1 pieces still gather
            h0_sched = {8: [(0, c) for c in range(3)],
                        9: [(0, c) for c in range(3, 6)],
                        10: [(0, c) for c in range(6, 8)] + [(1, 0)],
                        11: [(1, c) for c in range(1, 4)],
                        12: [(1, c) for c in range(4, 8)]}

            def h0_hook(g):
                for (j, c) in h0_sched.get(g, ()):
                    send_piece(0, j, c, 0, nc.scalar)
                    if (j, c) == (0, 7):
                        a2a(0, 0)
                if g == 12:
                    a2a(0, 1)

            layer(0, None, 0, mid_hook=h0_hook)
            for j in range(SCELL // PIECE):
                for c in range(NCORES):
                    send_piece(1, j, c, 2,
                               nc.sync if c % 2 == 0 else nc.scalar)
                a2a(1, j)
            for w in range(10):
                build_sl2(w)
            layer(1, h1T_sb, 2)
            if dbg:
                for h in range(2):
                    nc.sync.dma_start(h1dbg[h], h1own[h][:])
                nc.sync.dma_start(t2dbg[:], t2[:])
    nc.compile()
    return nc


# -------------------------------------------------------------------- host

def kernel(**inputs):
    from concourse.bass_utils import run_bass_kernel_spmd

    x = np.asarray(inputs["x"], dtype=np.float32)
    ei = np.asarray(inputs["edge_index"], dtype=np.int64)
    et = np.asarray(inputs["edge_type"], dtype=np.int64)
    src, dst = ei[0], ei[1]
    cnt = np.bincount(dst * R + et, minlength=N * R)
    norm = (1.0 / np.maximum(cnt[dst * R + et], 1)).astype(np.float32)

    import os
    p = _plan(src, dst, et, norm)
    dbg = bool(os.environ.get("BASS_DEBUG_STAGE"))
    nc = _build(dbg=dbg)

    x16 = x.astype(BF16)
    w1 = np.asarray(inputs["W1"], np.float32).astype(BF16)
    w2 = np.asarray(inputs["W2"], np.float32).astype(BF16)
    r1 = np.asarray(inputs["root1"], np.float32).astype(BF16)
    r2 = np.asarray(inputs["root2"], np.float32).astype(BF16)
    wp = np.asarray(inputs["Wp"], np.float32)
    b1 = np.asarray(inputs["b1"], np.float32)
    b2 = np.asarray(inputs["b2"], np.float32)
    bp = np.asarray(inputs["bp"], np.float32)

    w1s = np.ascontiguousarray(
        w1.reshape(R, 2, 128, 2, 128).transpose(2, 0, 1, 3, 4))
    w2s = np.ascontiguousarray(
        w2.reshape(R, 2, 128, 2, 128).transpose(2, 0, 1, 3, 4))
    r1s = np.ascontiguousarray(r1.reshape(2, 128, 2, 128).transpose(1, 0, 2, 3))
    r2s = np.ascontiguousarray(r2.reshape(2, 128, 2, 128).transpose(1, 0, 2, 3))
    wpu = np.ascontiguousarray(
        wp[:F].astype(BF16).reshape(2, 128, F).transpose(1, 0, 2))
    wpv = np.ascontiguousarray(
        wp[F:].astype(BF16).reshape(2, 128, F).transpose(1, 0, 2))
    b1c = np.ascontiguousarray(b1.reshape(2, 128).T)
    b2c = np.ascontiguousarray(b2.reshape(2, 128).T)

    inv = p["inv"]
    in_maps = []
    for c in range(NCORES):
        sn = p["stream_srcn"][c]
        ei_ = p["stream_eid"][c]
        msg = np.zeros((SLOTS, F), dtype=np.float32)
        valid = sn >= 0
        msg[valid] = x[sn[valid]] * norm[ei_[valid]][:, None]
        t1msg = np.ascontiguousarray(
            msg.astype(BF16).reshape(SLOTS // 128, 128, F)
            .transpose(1, 0, 2))

        xc = np.zeros((NCAP, F), dtype=BF16)
        gsel = np.arange(c * NCAP, (c + 1) * NCAP)
        f = p["filled"][gsel]
        xc[f] = x16[inv[gsel[f]]]
        xt = np.ascontiguousarray(xc.reshape(NCAP, 2, 128).transpose(2, 1, 0))
        in_maps.append({
            "t1msg": t1msg, "xt": xt,
            "w1s": w1s, "w2s": w2s, "r1s": r1s, "r2s": r2s,
            "wpu": wpu, "wpv": wpv, "b1c": b1c, "b2c": b2c,
            "mi": p["msg_idx"][c], "dstl": p["dstl"][c], "nrm": p["nrm"][c],
            "iota": np.tile(np.arange(128, dtype=np.int32), (128, 1)),
            "ci": p["cell_idx"][c],
        })

    res = None
    if os.environ.get("BASS_KERNEL_TRACE"):
        try:
            tdir = os.environ.get("BASS_KERNEL_TRACE_DIR") or None
            if tdir:
                os.makedirs(tdir, exist_ok=True)
            res = run_bass_kernel_spmd(nc, in_maps,
                                       core_ids=list(range(NCORES)),
                                       trace=True, tmpdir=tdir)
        except Exception:
            import traceback
            traceback.print_exc()
            res = None
    if res is None:
        res = run_bass_kernel_spmd(nc, in_maps, core_ids=list(range(NCORES)))
    global LAST_EXEC_NS, LAST_RES, LAST_PLAN
    LAST_EXEC_NS = res.exec_time_ns
    LAST_RES, LAST_PLAN = res, p

    core_of, win_of, slot_of = p["core_of"], p["win_of"], p["slot_of"]
    u_all = np.zeros((N, F), dtype=np.float32)
    v_all = np.zeros((N, F), dtype=np.float32)
    for c in range(NCORES):
        uo = np.asarray(res.results[c]["uo"]).astype(np.float32)
        vo = np.asarray(res.results[c]["vo"]).astype(np.float32)
        u256 = uo.transpose(1, 0, 2).reshape(F, NCAP)
        v256 = vo.transpose(1, 0, 2).reshape(F, NCAP)
        sel = np.where(core_of == c)[0]
        cols = win_of[sel] * 128 + slot_of[sel]
        u_all[sel] = u256[:, cols].T
        v_all[sel] = v256[:, cols].T
    return u_all[src] + v_all[dst] + bp[None, :]
